# revision 54
# baseline (speedup 1.0000x reference)
"""BitLinear forward on 8 Trainium2 NeuronCores (raw Bass implementation).

Math (reference, with EPS-clamped per-token scale xs = clip(mean|x|, EPS)):
    out = ((x / xs) @ sign(w).T + bias) * mean|w| * xs * scale
        = (x @ sign(w).T) * (mean|w| * scale) + bias * (mean|w| * scale * xs)

The xs normalize/denormalize cancels exactly on the matmul term (clamp
included), so the heavy path is a sign-binarized matmul scaled by the scalar
c = mean|w| * scale.  The bias term (zero for the graded input) is folded in
on the host when bias != 0.

Distribution: pure data-parallel over the 8192 tokens -- each of the 8 cores
computes 1024 rows against the full (replicated) weight.  No collectives;
mean|w| is computed redundantly per core.

Precision: x is cast to fp16 on the host (single pass; quantization error
~2e-4 relative l2, far under the 2e-2 gate -- a hi/lo split would double the
PE train for nothing).  w ships once as fp8 e4m3 with a sign-underflow fix
(|w| < 2^-10 would round to 0 and drop the sign, which alone costs ~3e-2
error); sign() on device is exact, and mean|w8| differs from mean|w| by
~7e-4 relative, which dominates the final ~7e-4 error -- still 28x under
the gate.  Shipping pre-signed fp16 tiles instead was tried and is slower:
2x the critical DMA bytes on rings measured at 115-220 GB/s beats the ACT
sign stage only if the sign stage is serial -- so here it is parallelized
(ACT signs tiles 0,2,4..15; DVE signs tiles 1,3 via a mult/min/max clamp).

The toolchain's walrus allows only ONE sync-wait per engine instruction,
which rules out the Tile scheduler, so the kernel is raw Bass: five explicit
engine programs synced by explicit semaphores.  Distinct DMA completions are
UNORDERED even on one ring, so every tile/slab has its own semaphore (a
counting sem would let "t+1 tiles done" pass while tile t is in flight --
this exact race produced intermittent NaNs on hardware).

Layout: x and w are pre-arranged on the host so every DMA is a pure linear
copy (1-4 KB contiguous per partition; strided DMA runs ~3x slower).  The
sign output w16 is tile-major so both the sign writes and the PE moving
slices stay trivially linear.

Engine schedule per core (rows=1024, k=2048, o=2048):
  SP  : w8 tiles 0-3 + x slab 0 quarters (startup critical, interleaved),
        x slabs 1-7, then output DMAs (8 MB)
  ACT : act-table preload (dummy sign on a memset scratch), sign tiles
        0,2,4..15 (w8 tiles 4-15 ride ACT's own HW ring, triggers hidden
        between signs), PSUM evictions interleaved
  DVE : sign tiles 1,3 (parallel with ACT -> column 0 ready ~4.8us
        earlier), |w8| row-sums, c reduction chain, outsb *= c (the only
        c-gated stage, so c latency never stalls PE or PSUM recycling)
  PE  : 14 warm-up matmuls on a memset scratch (HAM clock) ending as the
        first signs land, then 32 blocks x 16 matmuls at the 216 ns/MM
        N=512 fp16 issue floor; PSUM bank = row-block, column-major order
  POOL: scratch memsets + c-scalar DMA round trips

PE train: 512 MMs x 216 ns = 110.6 us (the fp16 issue floor = 100% of the
78.6 TF/s bf16 peak; fp8 DoubleRow measures the same 216 ns/MM for 2x K
per MM but needs a hi/lo double pass at this error budget, so it ties fp16
exactly; an fp8 moving operand measures 259 ns/MM -- see dr_bench2.py).
"""

import sys

sys.path.insert(0, "/opt/trn_rl_repo")

from contextlib import ExitStack

import ml_dtypes
import numpy as np

import concourse.bass as bass
import concourse.mybir as mybir

F32 = mybir.dt.float32
F16 = mybir.dt.float16
F8 = mybir.dt.float8e4
AF = mybir.ActivationFunctionType
ALU = mybir.AluOpType
AX = mybir.AxisListType

N_CORES = 8
EPS = 1e-5
P = 128
NT = 512          # output free-dim tile


def build_nc(rows, k, o):
    """Per-core kernel: out[rows, o] = (x_shard @ sign(w).T) * c.

    xt:  [n_m, 128, k]     f16  (x slab-linearized, see _linearize_x)
    wt:  [n_wt, 128, 4*NT] f8e4 (w tile-linearized, see _linearize_w)
    sc:  [1, 1]            f32  (scale)
    out: [rows, o]         f32
    """
    n_m = rows // P          # row blocks (8)
    n_n = o // NT            # output column blocks (4)
    n_ks = k // P            # K subtiles (16)
    n_wkt = k // NT          # w tiles per output column (4)
    n_wt = n_wkt * n_n       # w tiles of [128, ksub*NT] (16)
    n_blk = n_n * n_m        # output blocks (32)
    ksub = n_ks // n_wkt     # K subtiles per w tile (4)

    nc = bass.Bass()
    xt = nc.declare_dram_parameter("xt", [n_m, P, k], F16, isOutput=False)
    wt = nc.declare_dram_parameter("wt", [n_wt, P, ksub * NT], F8,
                                   isOutput=False)
    # column 0's sign tiles pre-computed as +-1 fp16: block 0 is then
    # purely DMA-paced instead of waiting on a serial sign chain
    ws = nc.declare_dram_parameter("ws", [n_wkt, P, ksub * NT], F16,
                                   isOutput=False)
    sc = nc.declare_dram_parameter("sc", [1, 1], F32, isOutput=False)
    out = nc.declare_dram_parameter("out", [rows, o], F32, isOutput=True)
    scr_col = nc.dram_tensor("scr_col", [P], F32)
    scr_c = nc.dram_tensor("scr_c", [1, 1], F32)

    out_ap = out[:, :].rearrange("(po pi) f -> pi po f", pi=P)  # [128, n_m, o]

    with ExitStack() as es:
        sem = lambda name: es.enter_context(nc.semaphore(name))
        sb = lambda name, shape, dt=F32: es.enter_context(
            nc.sbuf_tensor(name, shape, dt)
        )
        ps = lambda name: es.enter_context(nc.psum_tensor(name, [P, NT], F32))

        s_scs = sem("s_scs")      # scale scalar DMA
        # distinct DMA completions are UNORDERED (even on one ring): every
        # slab/tile gets its own semaphore
        s_xdma = [sem(f"s_xdma{m}") for m in range(n_m)]
        s_x0 = [sem(f"s_x0_{q}") for q in range(n_wkt)]  # slab-0 quarters
        s_w8 = [sem(f"s_w8_{t}") for t in range(n_wt)]
        s_ws = [sem(f"s_ws_{t}") for t in range(n_wkt)]  # col-0 sign DMAs
        s_warm = sem("s_warm")    # xwarm memset done
        s_pre = sem("s_pre")      # wpre memset done (ACT table preload)
        s_signA = sem("s_signA")  # ACT signs: t4..t15 (counts 1..12)
        s_wabs = sem("s_wabs")    # DVE |w8| row-sum of tile t done (1/tile)
        s_mm = sem("s_mm")        # PE finished block (1/block)
        s_evict = sem("s_evict")  # ACT finished evict (1/block)
        s_scaled = sem("s_scaled")  # DVE finished *c (1/block)
        s_odma = [sem(f"s_odma{i}") for i in range(n_m)]
        s_col = sem("s_col")      # DVE col reduce done
        s_c0 = sem("s_c0")        # col->dram dma
        s_c1 = sem("s_c1")        # dram->rowt dma
        s_dvec = sem("s_dvec")    # DVE c-chain step counter
        s_cts = sem("s_cts")      # DVE c scalar ready
        s_c2 = sem("s_c2")        # cts->dram dma
        s_cdma = sem("s_cdma")    # cb broadcast dma

        xhi = sb("xhi", [P, n_m, k], F16)        # 32 KB/partition
        xwarm = sb("xwarm", [P, NT + P], F16)    # prewarm dummy operands
        wpre = sb("wpre", [P, 8], F8)            # ACT table-preload scratch
        # sign tiles tile-major: sign writes and PE moving slices both
        # stay linear; tile t covers K-subtiles [kt*ksub, kt*ksub+ksub)
        # of output column nt (t = nt*n_wkt + kt)
        w16 = sb("w16", [P, n_wt, ksub * NT], F16)   # 64 KB/partition
        w8sb = sb("w8sb", [P, n_wt, ksub * NT], F8)  # 32 KB/partition
        acc = sb("acc", [P, n_wt], F32)
        outsb = sb("outsb", [P, n_m, NT], F32)   # 16 KB/partition
        scs = sb("scs", [1, 1], F32)
        col = sb("col", [P, 1], F32)
        rowt = sb("rowt", [1, P], F32)
        tot = sb("tot", [1, 1], F32)
        cts = sb("cts", [1, 1], F32)
        cb = sb("cb", [P, 1], F32)
        psum = [ps(f"psum{m}") for m in range(n_m)]

        # moving operand for (ks, nt): tile t = nt*n_wkt + ks//ksub,
        # sub-slice po = ks % ksub
        def w_mv(ks, nt):
            t = nt * n_wkt + ks // ksub
            po = ks % ksub
            return w16[:, t, po * NT : (po + 1) * NT]

        with nc.Block() as block:

            @block.sync
            def _(sp):
                # startup-critical interleave on the SP HW ring: col-0
                # sign tiles (gate block 0, DMA-paced) + x slab 0 quarters
                qs = k // n_wkt
                sp.dma_start(out=w16[:, 0], in_=ws[0]).then_inc(
                    s_ws[0], 16
                )
                for q in range(n_wkt):
                    sp.dma_start(
                        out=xhi[:, 0, q * qs : (q + 1) * qs],
                        in_=xt[0][:, q * qs : (q + 1) * qs],
                    ).then_inc(s_x0[q], 16)
                    if 1 + q < n_wkt:
                        sp.dma_start(
                            out=w16[:, 1 + q], in_=ws[1 + q]
                        ).then_inc(s_ws[1 + q], 16)
                sp.dma_start(out=scs[:], in_=sc[:, :]).then_inc(s_scs, 16)
                # x slabs 1-7 interleaved with the w8 magnitude tiles that
                # ride SP (0-3: abs only; 12-15: abs + ACT signs for col 3,
                # deadline ~90us).  Slab m is needed at ~22+3.46(m-1) us;
                # the w8 tiles feed the |w8|->c chain whose deadline is the
                # first outsb-ring reuse (~57us) -- neither may starve the
                # other, hence the interleave.  Tiles 4-11 ride ACT's ring.
                w8_sp = list(range(min(4, n_wt)))
                order = []
                for m in range(1, n_m):
                    order.append(("x", m))
                    if m >= 3 and w8_sp:
                        order.append(("w", w8_sp.pop(0)))
                order += [("w", t) for t in w8_sp]
                for kind, i in order:
                    if kind == "x":
                        sp.dma_start(out=xhi[:, i], in_=xt[i]).then_inc(
                            s_xdma[i], 16
                        )
                    else:
                        sp.dma_start(out=w8sb[:, i], in_=wt[i]).then_inc(
                            s_w8[i], 16
                        )
                # output DMAs (SP HW ring is free from ~25us on)
                for idx in range(n_blk):
                    nt, m = divmod(idx, n_m)
                    sp.wait_ge(s_scaled, idx + 1)
                    sp.dma_start(
                        out=out_ap[:, m, nt * NT : (nt + 1) * NT],
                        in_=outsb[:, idx % n_m],
                    ).then_inc(s_odma[idx % n_m], 16)

            @block.scalar
            def _(act):
                def dma_w(t):
                    act.dma_start(out=w8sb[:, t], in_=wt[t]).then_inc(
                        s_w8[t], 16
                    )

                def evict(j):
                    nt, m = divmod(j, n_m)
                    act.wait_ge(s_mm, j + 1)
                    if j >= n_m:
                        act.wait_ge(s_odma[j % n_m], 16 * (j // n_m))
                    act.copy(outsb[:, j % n_m], psum[m][:]).then_inc(
                        s_evict, 1
                    )

                def sign(t):
                    act.wait_ge(s_w8[t], 16)
                    act.activation(
                        w16[:, t], w8sb[:, t], AF.Sign
                    ).then_inc(s_signA, 1)

                # force the activation-table load NOW (it is inserted
                # before the first ACTIVATE; a dummy on memset scratch
                # keeps it off the DMA-wait critical path)
                act.wait_ge(s_pre, 1)
                act.activation(wpre[:, 0:4], wpre[:, 4:8], AF.Sign)
                # ACT's own ring carries w8 tiles 4..11 (cols 1-2)
                dma_w(4)
                dma_w(5)
                evict_count = 0
                for t in range(4, n_wt):
                    if 6 <= t + 2 < n_wt:
                        dma_w(t + 2)
                    sign(t)
                    # interleave evictions only once their s_mm waits are
                    # already satisfied -- an early evict would head-of-
                    # line-block the sign pipeline (signs went block-paced
                    # when evicts started at t=4)
                    if t >= 8 and evict_count < n_blk:
                        evict(evict_count)
                        evict_count += 1
                for j in range(evict_count, n_blk):
                    evict(j)

            @block.vector
            def _(dve):
                # |w8| row-sums per tile; c is only needed by the *c stage
                # (which lags evictions by design), so latency here is free
                for t in range(n_wt):
                    dve.wait_ge(s_w8[t], 16)
                    dve.tensor_reduce(
                        acc[:, t : t + 1], w8sb[:, t], axis=AX.X,
                        op=ALU.add, apply_absolute_value=True,
                    ).then_inc(s_wabs, 1)
                # c chain: sum|w| -> scalar c (cross-partition via DMA
                # round trips on POOL)
                dve.wait_ge(s_scs, 16)
                dve.wait_ge(s_wabs, n_wt)
                dve.tensor_reduce(
                    col[:], acc[:], axis=AX.X, op=ALU.add
                ).then_inc(s_col, 1)
                dve.wait_ge(s_c1, 16)
                dve.tensor_reduce(
                    tot[:], rowt[:], axis=AX.X, op=ALU.add
                ).then_inc(s_dvec, 1)
                dve.wait_ge(s_dvec, 1)
                dve.tensor_tensor(
                    out=cts[:], in0=tot[:], in1=scs[:], op=ALU.mult
                ).then_inc(s_dvec, 1)
                dve.wait_ge(s_dvec, 2)
                dve.tensor_scalar(
                    cts[:], cts[:], 1.0 / (k * o), None, ALU.mult
                ).then_inc(s_cts, 1)
                # outsb scaling: out_sb *= c
                dve.wait_ge(s_cdma, 16)
                for idx in range(n_blk):
                    dve.wait_ge(s_evict, idx + 1)
                    dve.tensor_scalar(
                        outsb[:, idx % n_m],
                        outsb[:, idx % n_m],
                        cb[:],
                        None,
                        ALU.mult,
                    ).then_inc(s_scaled, 1)

            @block.tensor
            def _(pe):
                # spin the HAM activity window on a memset scratch until
                # the first sign tile lands; 8 cold matmuls at ~427ns
                # hand off to the DMA-paced block 0
                pe.wait_ge(s_warm, 1)
                for i in range(8):
                    pe.matmul(
                        psum[0][:],
                        xwarm[:, NT : NT + P],
                        xwarm[:, 0:NT],
                        start=(i == 0),
                        stop=(i == 7),
                    )
                for idx in range(n_blk):
                    nt, m = divmod(idx, n_m)
                    if m > 0:
                        pe.wait_ge(s_xdma[m], 16)
                    elif idx > 0:
                        # first block of column nt>=1: tiles 4nt..4nt+3
                        # are ACT signs 4(nt-1)+1 .. 4nt
                        pe.wait_ge(s_signA, n_wkt * nt)
                    if nt >= 1:
                        pe.wait_ge(s_evict, (nt - 1) * n_m + m + 1)
                    last = None
                    for ks in range(n_ks):
                        if idx == 0 and ks % ksub == 0:
                            # block 0 chases the startup DMAs tile-by-tile
                            kt = ks // ksub
                            pe.wait_ge(s_x0[kt], 16)
                            pe.wait_ge(s_ws[kt], 16)
                        last = pe.matmul(
                            psum[m][:],
                            xhi[:, m, ks * P : (ks + 1) * P],
                            w_mv(ks, nt),
                            start=(ks == 0),
                            stop=(ks == n_ks - 1),
                        )
                    last.then_inc(s_mm, 1)

            @block.gpsimd
            def _(gp):
                gp.memset(wpre[:], 1.0).then_inc(s_pre, 1)
                gp.memset(xwarm[:], 0.25).then_inc(s_warm, 1)
                # c-scalar DMA round trips (SW ring; idle until needed)
                gp.wait_ge(s_col, 1)
                gp.dma_start(out=scr_col[:], in_=col[:, 0]).then_inc(s_c0, 16)
                gp.wait_ge(s_c0, 16)
                gp.dma_start(out=rowt[:], in_=scr_col[None, :]).then_inc(
                    s_c1, 16
                )
                gp.wait_ge(s_cts, 1)
                gp.dma_start(out=scr_c[:, :], in_=cts[:]).then_inc(s_c2, 16)
                gp.wait_ge(s_c2, 16)
                gp.dma_start(
                    out=cb[:], in_=scr_c[:, :].to_broadcast([P, 1])
                ).then_inc(s_cdma, 16)

    return nc


def _linearize_x(shard, n_m, n_ks):
    # shard [rows, k] -> fp16 [n_m, P(pi), n_ks*P] with per-partition-linear
    # slabs: elem (m, pi, po*P + r) = shard[m*P + r, po*P + pi]
    a = shard.reshape(n_m, P, n_ks, P)          # (m, r, po, pi)
    b = np.ascontiguousarray(a.transpose(0, 3, 2, 1)).reshape(n_m, P, -1)
    return b.astype(np.float16)


def _tile_w(arr, n_n, n_wkt, ksub):
    # [o, k] -> [n_wt, P(pi), ksub*NT] (tile t = nt*n_wkt + kt):
    # elem (t, pi, po*NT + oo) = arr[nt*NT + oo, (kt*ksub+po)*P + pi]
    a = arr.reshape(n_n, NT, n_wkt, ksub, P)     # (nt, oo, kt, po, pi)
    b = a.transpose(0, 2, 4, 3, 1)               # (nt, kt, pi, po, oo)
    return np.ascontiguousarray(b).reshape(n_n * n_wkt, P, ksub * NT)


def _linearize_w(weight, n_n, n_wkt, ksub):
    # w8: fp8e4m3 magnitude/sign tiles.  e4m3 underflow fix: |w| < 2^-10
    # rounds to 0 and would drop the sign; restore the minimum subnormal
    # with the sign preserved.  ws: column-0 sign tiles (+-1 fp16, exact)
    # so block 0 is DMA-paced rather than sign-paced.
    wh = weight.astype(ml_dtypes.float8_e4m3)
    flip = (wh == 0) & (weight != 0)
    if flip.any():
        tiny = np.float32(2.0 ** -9)
        wh[flip] = np.copysign(tiny, weight[flip]).astype(
            ml_dtypes.float8_e4m3
        )
    w8t = _tile_w(wh, n_n, n_wkt, ksub)
    ws0 = _tile_w(np.sign(weight).astype(np.float16), n_n, n_wkt,
                  ksub)[:n_wkt]
    return w8t, np.ascontiguousarray(ws0)


_NC_CACHE = {}


def _get_nc(rows, k, o):
    key = (rows, k, o)
    if key not in _NC_CACHE:
        _NC_CACHE[key] = build_nc(rows, k, o)
    return _NC_CACHE[key]


def _run(x, weight, bias, scale, trace=False, tmpdir=None):
    from concourse.bass_utils import run_bass_kernel_spmd

    x = np.asarray(x, dtype=np.float32)
    weight = np.asarray(weight, dtype=np.float32)
    bias_arr = np.asarray(bias, dtype=np.float32).reshape(-1)
    scale_arr = np.asarray(scale, dtype=np.float32).reshape(1, 1)

    b, s, d_in = x.shape
    d_out = weight.shape[0]
    rows_total = b * s
    rows = rows_total // N_CORES

    n_m = rows // P
    n_n = d_out // NT
    n_wkt = d_in // NT
    ksub = (d_in // P) // n_wkt

    nc = _get_nc(rows, d_in, d_out)

    x2 = x.reshape(rows_total, d_in)
    wlin, ws0 = _linearize_w(weight, n_n, n_wkt, ksub)
    in_maps = []
    for i in range(N_CORES):
        shard = x2[i * rows : (i + 1) * rows]
        in_maps.append({
            "xt": _linearize_x(shard, n_m, d_in // P),
            "wt": wlin,
            "ws": ws0,
            "sc": scale_arr,
        })

    res = run_bass_kernel_spmd(
        nc, in_maps, list(range(N_CORES)), trace=trace, tmpdir=tmpdir
    )
    out = np.concatenate([r["out"] for r in res.results], axis=0)
    out = out.reshape(b, s, d_out)

    if np.any(bias_arr):
        # bias term (zero for the graded input): out += bias * c * xs,
        # with c exactly as the device computed it (mean|w8| * scale)
        c = np.abs(wlin.astype(np.float32)).mean() * scale_arr.ravel()[0]
        xs = np.clip(np.abs(x).mean(axis=-1, keepdims=True), EPS, None)
        out = out + bias_arr[None, None, :] * (c * xs)
    return out, res


def kernel(x, weight, bias, scale):
    return _run(x, weight, bias, scale)[0]


# revision 55
# speedup vs baseline: 1.1787x; 1.1787x over previous
"""BitLinear forward on 8 Trainium2 NeuronCores (raw Bass implementation).

Math (reference, with EPS-clamped per-token scale xs = clip(mean|x|, EPS)):
    out = ((x / xs) @ sign(w).T + bias) * mean|w| * xs * scale
        = (x @ sign(w).T) * (mean|w| * scale) + bias * (mean|w| * scale * xs)

The xs normalize/denormalize cancels exactly on the matmul term (clamp
included), so the heavy path is a sign-binarized matmul scaled by the scalar
c = mean|w| * scale.  The bias term (zero for the graded input) is folded in
on the host when bias != 0.

Distribution: pure data-parallel over the 8192 tokens -- each of the 8 cores
computes 1024 rows against the full (replicated) weight.  No collectives;
mean|w| is computed redundantly per core.

Precision: x is cast to fp16 on the host (single pass; quantization error
~2e-4 relative l2, far under the 2e-2 gate -- a hi/lo split would double
the PE train for nothing).  w ships as fp8 e4m3 with a sign-underflow fix
(|w| < 2^-10 would round to 0 and drop the sign, which alone costs ~3e-2
error); sign() on device is then exact, and mean|w8| differs from mean|w|
by ~7e-4 relative, which dominates the final ~7e-4 error -- still 28x
under the 2e-2 gate.

The toolchain's walrus allows only ONE sync-wait per engine instruction,
which rules out the Tile scheduler, so the kernel is raw Bass: five explicit
engine programs synced by explicit semaphores.  Distinct DMA completions
are UNORDERED even on one ring, so every tile/slab gets its own semaphore
(a counting sem would let "t+1 tiles done" pass while tile t is still in
flight -- this exact race produced intermittent NaNs on hardware).

Layout: both x and w are pre-arranged on the host so every DMA is a pure
linear copy (1-4 KB contiguous per partition; strided DMA runs ~3x slower).

Engine schedule per core (rows=1024, k=2048, o=2048):
  SP  : x slab DMAs (fp16, 4 MB) + scale scalar, then output DMAs (8 MB)
  ACT : w8 tile DMAs on its own HW ring (4 MB), sign(w8) -> w16 fp16,
        PSUM evictions interleaved into the sign loop.  The |w8| tiles
        arriving early for signs also feed DVE's reduction, so c is ready
        by ~40us -- well before the outsb ring first recycles (~57us).
  DVE : |w8| row-sums per tile, c reduction chain, outsb *= c (the only
        c-gated stage, so c latency never stalls PE or PSUM recycling)
  PE  : 12 warm-up matmuls on x slab 0 (HAM clock), then 32 blocks x 16
        matmuls at the 216 ns/MM N=512 fp16 issue floor; PSUM bank =
        row-block, column-major block order
  POOL: c-scalar DMA round trips (cross-partition reduce + broadcast)

PE train: 512 MMs x 216 ns = 110.6 us (the fp16 issue floor = 100% of the
78.6 TF/s bf16 peak).  Alternatives measured and rejected: fp8 DoubleRow
runs the same 216 ns/MM for 2x K per MM but needs a hi/lo double pass at
this error budget, tying fp16 exactly; a single fp8 pass fails the gate
(2.7e-2); an fp8 moving operand runs 259 ns/MM; shipping pre-signed fp16
tiles doubles the startup-critical DMA bytes and repeatedly lost 5-25 us
to ring congestion and c-chain deadline misses (see dr_bench*.py and the
session trace notes).
"""

import sys

sys.path.insert(0, "/opt/trn_rl_repo")

from contextlib import ExitStack

import ml_dtypes
import numpy as np

import concourse.bass as bass
import concourse.mybir as mybir

F32 = mybir.dt.float32
F16 = mybir.dt.float16
F8 = mybir.dt.float8e4
AF = mybir.ActivationFunctionType
ALU = mybir.AluOpType
AX = mybir.AxisListType

N_CORES = 8
EPS = 1e-5
P = 128
NT = 512          # output free-dim tile


def build_nc(rows, k, o):
    """Per-core kernel: out[rows, o] = (x_shard @ sign(w).T) * c.

    xt:  [n_m, 128, k]     f16  (x slab-linearized, see _linearize_x)
    wt:  [n_wt, 128, 4*NT] f8e4 (w tile-linearized, see _linearize_w)
    sc:  [1, 1]            f32  (scale)
    out: [rows, o]         f32
    """
    n_m = rows // P          # row blocks (8)
    n_n = o // NT            # output column blocks (4)
    n_ks = k // P            # K subtiles (16)
    n_wkt = k // NT          # w tiles per output column (4)
    n_wt = n_wkt * n_n       # w tiles of [128, ksub*NT] (16)
    n_blk = n_n * n_m        # output blocks (32)
    ksub = n_ks // n_wkt     # K subtiles per w tile (4)

    nc = bass.Bass()
    xt = nc.declare_dram_parameter("xt", [n_m, P, k], F16, isOutput=False)
    wt = nc.declare_dram_parameter("wt", [n_wt, P, ksub * NT], F8,
                                   isOutput=False)
    sc = nc.declare_dram_parameter("sc", [1, 1], F32, isOutput=False)
    out = nc.declare_dram_parameter("out", [rows, o], F32, isOutput=True)
    scr_col = nc.dram_tensor("scr_col", [P], F32)
    scr_c = nc.dram_tensor("scr_c", [1, 1], F32)

    out_ap = out[:, :].rearrange("(po pi) f -> pi po f", pi=P)  # [128, n_m, o]

    with ExitStack() as es:
        sem = lambda name: es.enter_context(nc.semaphore(name))
        sb = lambda name, shape, dt=F32: es.enter_context(
            nc.sbuf_tensor(name, shape, dt)
        )
        ps = lambda name: es.enter_context(nc.psum_tensor(name, [P, NT], F32))

        s_scs = sem("s_scs")      # scale scalar DMA
        # distinct DMA completions are UNORDERED (even on one ring): a
        # counting sem would let "t+1 tiles done" pass while tile t is
        # still in flight, so every slab/tile gets its own semaphore
        s_xdma = [sem(f"s_xdma{m}") for m in range(n_m)]
        s_w8 = [sem(f"s_w8_{t}") for t in range(n_wt)]
        s_sign = sem("s_sign")    # ACT sign of tile t done (1/tile)
        s_wabs = sem("s_wabs")    # DVE |w8| row-sum of tile t done (1/tile)
        s_mm = sem("s_mm")        # PE finished block (1/block)
        s_evict = sem("s_evict")  # ACT finished evict (1/block)
        s_scaled = sem("s_scaled")  # DVE finished *c (1/block)
        s_odma = [sem(f"s_odma{i}") for i in range(n_m)]
        s_col = sem("s_col")      # DVE col reduce done
        s_c0 = sem("s_c0")        # col->dram dma
        s_c1 = sem("s_c1")        # dram->rowt dma
        s_dvec = sem("s_dvec")    # DVE c-chain step counter
        s_cts = sem("s_cts")      # DVE c scalar ready
        s_c2 = sem("s_c2")        # cts->dram dma
        s_cdma = sem("s_cdma")    # cb broadcast dma

        xhi = sb("xhi", [P, n_m, k], F16)        # 32 KB/partition
        w8sb = sb("w8sb", [P, n_wt, ksub * NT], F8)  # 32 KB/partition
        w16 = sb("w16", [P, n_ks, o], F16)       # 64 KB/partition
        acc = sb("acc", [P, n_wt], F32)
        outsb = sb("outsb", [P, n_m, NT], F32)   # 16 KB/partition
        scs = sb("scs", [1, 1], F32)
        col = sb("col", [P, 1], F32)
        rowt = sb("rowt", [1, P], F32)
        tot = sb("tot", [1, 1], F32)
        cts = sb("cts", [1, 1], F32)
        cb = sb("cb", [P, 1], F32)
        psum = [ps(f"psum{m}") for m in range(n_m)]

        # w tile order: n-major (all k-tiles of column 0 first), so early
        # signs unlock output column 0 for the PE
        w_order = [(kt, nt) for nt in range(n_n) for kt in range(n_wkt)]

        with nc.Block() as block:

            @block.sync
            def _(sp):
                # x slab 0 first: it gates PE warm-up + block 0
                for m in range(n_m):
                    sp.dma_start(out=xhi[:, m], in_=xt[m]).then_inc(
                        s_xdma[m], 16
                    )
                    if m == 0:
                        sp.dma_start(out=scs[:], in_=sc[:, :]).then_inc(
                            s_scs, 16
                        )
                # output DMAs (SP HW ring is idle from here on)
                for idx in range(n_blk):
                    nt, m = divmod(idx, n_m)
                    sp.wait_ge(s_scaled, idx + 1)
                    sp.dma_start(
                        out=out_ap[:, m, nt * NT : (nt + 1) * NT],
                        in_=outsb[:, idx % n_m],
                    ).then_inc(s_odma[idx % n_m], 16)

            @block.scalar
            def _(act):
                # w8 DMAs on the Scalar HW ring, self-paced; signs follow
                # the ring, evictions interleave once their s_mm wait is
                # near.
                def dma_w(t):
                    act.dma_start(out=w8sb[:, t], in_=wt[t]).then_inc(
                        s_w8[t], 16
                    )

                def evict(j):
                    nt, m = divmod(j, n_m)
                    act.wait_ge(s_mm, j + 1)
                    if j >= n_m:
                        act.wait_ge(s_odma[j % n_m], 16 * (j // n_m))
                    act.copy(outsb[:, j % n_m], psum[m][:]).then_inc(
                        s_evict, 1
                    )

                evict_count = 0
                for t in range(min(3, n_wt)):
                    dma_w(t)
                for t in range(n_wt):
                    if 3 <= t + 3 < n_wt:
                        dma_w(t + 3)
                    kt, nt = w_order[t]
                    act.wait_ge(s_w8[t], 16)
                    act.activation(
                        w16[:, kt * ksub : (kt + 1) * ksub,
                            nt * NT : (nt + 1) * NT],
                        w8sb[:, t],
                        AF.Sign,
                    ).then_inc(s_sign, 1)
                    # interleave early evictions (block j completes ~3.5us
                    # apart; placing evict j after sign j+5 keeps the s_mm
                    # wait short without stalling the sign pipeline)
                    if t >= 5 and evict_count < n_blk:
                        evict(evict_count)
                        evict_count += 1
                for j in range(evict_count, n_blk):
                    evict(j)

            @block.vector
            def _(dve):
                # |w8| row-sums per tile (c is only needed by the *c stage,
                # which lags evictions, so this never gates PE)
                for t in range(n_wt):
                    dve.wait_ge(s_w8[t], 16)
                    dve.tensor_reduce(
                        acc[:, t : t + 1], w8sb[:, t], axis=AX.X,
                        op=ALU.add, apply_absolute_value=True,
                    ).then_inc(s_wabs, 1)
                # c chain: sum|w| -> scalar c (cross-partition via DMA
                # round trips on POOL)
                dve.wait_ge(s_scs, 16)
                dve.wait_ge(s_wabs, n_wt)
                dve.tensor_reduce(
                    col[:], acc[:], axis=AX.X, op=ALU.add
                ).then_inc(s_col, 1)
                dve.wait_ge(s_c1, 16)
                dve.tensor_reduce(
                    tot[:], rowt[:], axis=AX.X, op=ALU.add
                ).then_inc(s_dvec, 1)
                dve.wait_ge(s_dvec, 1)
                dve.tensor_tensor(
                    out=cts[:], in0=tot[:], in1=scs[:], op=ALU.mult
                ).then_inc(s_dvec, 1)
                dve.wait_ge(s_dvec, 2)
                dve.tensor_scalar(
                    cts[:], cts[:], 1.0 / (k * o), None, ALU.mult
                ).then_inc(s_cts, 1)
                # outsb scaling: out_sb *= c
                dve.wait_ge(s_cdma, 16)
                for idx in range(n_blk):
                    dve.wait_ge(s_evict, idx + 1)
                    dve.tensor_scalar(
                        outsb[:, idx % n_m],
                        outsb[:, idx % n_m],
                        cb[:],
                        None,
                        ALU.mult,
                    ).then_inc(s_scaled, 1)

            @block.tensor
            def _(pe):
                prewarm = n_m >= 1 and k >= 2 * NT and rows >= P
                if prewarm:
                    # spin the HAM activity window while DMAs land; reads
                    # only x slab 0 (already complete), results discarded
                    pe.wait_ge(s_xdma[0], 16)
                    for i in range(12):
                        pe.matmul(
                            psum[0][:],
                            xhi[:, 0, NT : NT + P],
                            xhi[:, 0, 0:NT],
                            start=(i == 0),
                            stop=(i == 11),
                        )
                for idx in range(n_blk):
                    nt, m = divmod(idx, n_m)
                    pe.wait_ge(s_xdma[m], 16)
                    if idx > 0:
                        pe.wait_ge(s_sign, n_wkt * (nt + 1))
                    if nt >= 1:
                        pe.wait_ge(s_evict, (nt - 1) * n_m + m + 1)
                    last = None
                    for ks in range(n_ks):
                        if idx == 0 and ks % ksub == 0:
                            # block 0 starts as soon as its first w tiles
                            # are signed
                            pe.wait_ge(s_sign, ks // ksub + 1)
                        last = pe.matmul(
                            psum[m][:],
                            xhi[:, m, ks * P : (ks + 1) * P],
                            w16[:, ks, nt * NT : (nt + 1) * NT],
                            start=(ks == 0),
                            stop=(ks == n_ks - 1),
                        )
                    last.then_inc(s_mm, 1)

            @block.gpsimd
            def _(gp):
                # c-scalar DMA round trips (SW ring; idle until needed)
                gp.wait_ge(s_col, 1)
                gp.dma_start(out=scr_col[:], in_=col[:, 0]).then_inc(s_c0, 16)
                gp.wait_ge(s_c0, 16)
                gp.dma_start(out=rowt[:], in_=scr_col[None, :]).then_inc(
                    s_c1, 16
                )
                gp.wait_ge(s_cts, 1)
                gp.dma_start(out=scr_c[:, :], in_=cts[:]).then_inc(s_c2, 16)
                gp.wait_ge(s_c2, 16)
                gp.dma_start(
                    out=cb[:], in_=scr_c[:, :].to_broadcast([P, 1])
                ).then_inc(s_cdma, 16)

    return nc


def _linearize_x(shard, n_m, n_ks):
    # shard [rows, k] -> fp16 [n_m, P(pi), n_ks*P] with per-partition-linear
    # slabs: elem (m, pi, po*P + r) = shard[m*P + r, po*P + pi]
    a = shard.reshape(n_m, P, n_ks, P)          # (m, r, po, pi)
    b = np.ascontiguousarray(a.transpose(0, 3, 2, 1)).reshape(n_m, P, -1)
    return b.astype(np.float16)


def _linearize_w(weight, n_n, n_wkt, ksub):
    # weight [o, k] -> fp8e4m3 [n_wt, P(pi), ksub*NT] (tile t = nt*n_wkt+kt):
    # elem (t, pi, po*NT + oo) = weight[nt*NT + oo, (kt*ksub+po)*P + pi].
    # e4m3 quarters the w DMA vs f32; sign() stays exact thanks to the
    # underflow fix, and mean|w| moves by ~7e-4 relative.
    wh = weight.astype(ml_dtypes.float8_e4m3)
    flip = (wh == 0) & (weight != 0)  # underflowed-to-zero: keep the sign
    if flip.any():
        tiny = np.float32(2.0 ** -9)  # e4m3 min subnormal
        wh[flip] = np.copysign(tiny, weight[flip]).astype(
            ml_dtypes.float8_e4m3
        )
    a = wh.reshape(n_n, NT, n_wkt, ksub, P)      # (nt, oo, kt, po, pi)
    b = a.transpose(0, 2, 4, 3, 1)               # (nt, kt, pi, po, oo)
    return np.ascontiguousarray(b).reshape(n_n * n_wkt, P, ksub * NT)


_NC_CACHE = {}


def _get_nc(rows, k, o):
    key = (rows, k, o)
    if key not in _NC_CACHE:
        _NC_CACHE[key] = build_nc(rows, k, o)
    return _NC_CACHE[key]


def _run(x, weight, bias, scale, trace=False, tmpdir=None):
    from concourse.bass_utils import run_bass_kernel_spmd

    x = np.asarray(x, dtype=np.float32)
    weight = np.asarray(weight, dtype=np.float32)
    bias_arr = np.asarray(bias, dtype=np.float32).reshape(-1)
    scale_arr = np.asarray(scale, dtype=np.float32).reshape(1, 1)

    b, s, d_in = x.shape
    d_out = weight.shape[0]
    rows_total = b * s
    rows = rows_total // N_CORES

    n_m = rows // P
    n_n = d_out // NT
    n_wkt = d_in // NT
    ksub = (d_in // P) // n_wkt

    nc = _get_nc(rows, d_in, d_out)

    x2 = x.reshape(rows_total, d_in)
    wlin = _linearize_w(weight, n_n, n_wkt, ksub)
    in_maps = []
    for i in range(N_CORES):
        shard = x2[i * rows : (i + 1) * rows]
        in_maps.append({
            "xt": _linearize_x(shard, n_m, d_in // P),
            "wt": wlin,
            "sc": scale_arr,
        })

    res = run_bass_kernel_spmd(
        nc, in_maps, list(range(N_CORES)), trace=trace, tmpdir=tmpdir
    )
    out = np.concatenate([r["out"] for r in res.results], axis=0)
    out = out.reshape(b, s, d_out)

    if np.any(bias_arr):
        # bias term (zero for the graded input): out += bias * c * xs,
        # with c exactly as the device computed it (mean|w8| * scale)
        c = np.abs(wlin.astype(np.float32)).mean() * scale_arr.ravel()[0]
        xs = np.clip(np.abs(x).mean(axis=-1, keepdims=True), EPS, None)
        out = out + bias_arr[None, None, :] * (c * xs)
    return out, res


def kernel(x, weight, bias, scale):
    return _run(x, weight, bias, scale)[0]
